# revision 13
# baseline (speedup 1.0000x reference)
"""BinASPP Trainium2 kernel (Bass/Tile), SPMD over 8 NeuronCores.

Strategy (v3)
-------------
Data-parallel over batch: N=8 images -> 1 image per core.  binarize()
forward == sign(), so every conv is a matmul over {-1,+1} values: exact in
fp8e4 with fp32 PSUM accumulation.  A dilated 3x3 conv is 9 shifted 1x1
convs (taps) over one zero-padded sign image (pad 12, 88x88) resident in
SBUF; each DoubleRow matmul contracts all K=256 input channels and streams
a 4D access pattern [k2, 8 rows, 64 cols] -- only useful columns, no pad
streaming (v2 streamed full padded rows: ~20% wasted PE columns).  8-row
tiles fill one 2KB PSUM bank exactly.

Engine split:
 - ACT: x sign passes + plain [128,512] PSUM->SBUF fp16 drains (y fp16 is
   exact: even integers, |y| <= 2304).
 - DVE: per-tile bn_stats (512-elem chunks) immediately after each drain,
   so a branch's stats are one bn_aggr away when its last tile lands;
   clip+scale applies u_j = a_j*clip(y_j) with the branch sum accumulated
   in place via scalar_tensor_tensor (q += a*clip(y)); late-group moment
   merges.
 - Pool (gpsimd): pad memsets, pool-branch mean reduces, early-group
   moment merges (the Pool queue stalls through each pending collective's
   completion, so merges are placed in its free windows), collective
   triggers.
 - Sync-BN: per-group AllGather of (mean, var); each core merges the 8
   cores' moments locally.  Five groups -- {pool,1x1,r1} {r4} {r8}
   {r12 mc0} {r12 mc1} -- so only the final mc1 half-gather sits in the
   tail; mc0's output DMA ships inside that window.
BN offsets d_j (+ the pool branch's a*y_pool + d) fold into a per-channel
s0 applied in the tail passes; output is stored fp16 and widened on host.
"""

import numpy as np
import ml_dtypes
from contextlib import ExitStack

import concourse.bass as bass
import concourse.bacc as bacc
import concourse.mybir as mybir
import concourse.tile as tile
from concourse.bass_utils import run_bass_kernel_spmd

AF = mybir.ActivationFunctionType
ALU = mybir.AluOpType
AX = mybir.AxisListType
F32 = mybir.dt.float32
F16 = mybir.dt.float16
FP8 = mybir.dt.float8e4
DR = mybir.MatmulPerfMode.DoubleRow

P = 128
CIN = 256
COUT = 256
H = W = 64
HW = H * W
PAD = 12
PW = H + 2 * PAD              # 88
ILEN = PW * PW                # 7744 (multiple of 16 -> DR k-stride rule)
RATES = (1, 4, 8, 12)
NT = 2 + 9 * len(RATES)       # 38 tap matrices
NBLK1 = 22                    # lhsT blocks for pool/1x1/r1 (shipped first)
EPS = 1e-5
N_CORES = 8
TROWS = 8                     # rows per PSUM tile (8*64 = 512 f32 = 1 bank)
NTILE = H // TROWS            # 8 tiles per branch-mc
# branch ids: 0=pool, 1=1x1, 2=r1, 3=r4, 4=r8, 5=r12
BR = {1: (1, None), 2: (2, 1), 3: (11, 4), 4: (20, 8), 5: (29, 12)}
# stats slots (j, mc) grouped per AllGather, in gather order; G0's slots
# are mc-outer so its d-sum reduces map onto s0's [mc] layout.
GROUPS = [[(0, 0), (1, 0), (2, 0), (0, 1), (1, 1), (2, 1)],
          [(3, 0), (3, 1)], [(4, 0), (4, 1)], [(5, 0)], [(5, 1)]]
GOFF = [0, 6, 8, 10, 11]      # slot offset of each group in the gb pack
SLOT = {}
for _g, _members in enumerate(GROUPS):
    for _si, _jm in enumerate(_members):
        SLOT[_jm] = (_g, _si)
# x row-blocks: small first block so the first matmul starts early
XBLKS = [(0, 12), (12, 12), (24, 20), (44, 20)]


def build(n_cores: int = N_CORES):
    nc = bacc.Bacc(
        "TRN2",
        target_bir_lowering=False,
        debug=False,
        enable_asserts=False,
        num_devices=n_cores,
    )
    xs = nc.dram_tensor("xs", [CIN, H, W], F32, kind="ExternalInput")
    wt = nc.dram_tensor("wt", [P, NT * 2, 2, P], FP8, kind="ExternalInput")
    gb = nc.dram_tensor("gb", [P, 12, 2], F32, kind="ExternalInput")
    out = nc.dram_tensor("out", [COUT, H, W], F16, kind="ExternalOutput")

    with tile.TileContext(nc) as tc, ExitStack() as ctx:
        const = ctx.enter_context(tc.tile_pool(name="const", bufs=1))
        xload = ctx.enter_context(tc.tile_pool(name="xload", bufs=6))
        ppool = ctx.enter_context(
            tc.tile_pool(name="ppool", bufs=6, space=bass.MemorySpace.PSUM))
        psmall = ctx.enter_context(
            tc.tile_pool(name="psmall", bufs=2, space=bass.MemorySpace.PSUM))
        ybuf = ctx.enter_context(tc.tile_pool(name="ybuf", bufs=1))
        stat = ctx.enter_context(tc.tile_pool(name="stat", bufs=1))
        tmp = ctx.enter_context(tc.tile_pool(name="tmp", bufs=4))
        sbout = ctx.enter_context(tc.tile_pool(name="sbout", bufs=4))
        dram = ctx.enter_context(
            tc.tile_pool(name="dram", bufs=1, space=bass.MemorySpace.DRAM))

        lhsT = const.tile([P, NT * 2, 2, P], FP8, tag="lhsT")
        gb_sb = const.tile([P, 12, 2], F32, tag="gb")
        nc.scalar.dma_start(gb_sb[:], gb.ap())

        def wdr(blk):
            return lhsT[:, blk]          # [P, 2, P] fp8, k-interleaved

        # ---- padded k-interleaved sign image.  Pad zeroing: top pad on
        # DVE (gates r1 tile 0), seams/bottom on Pool interleaved with
        # the kc1 x-load triggers.
        sxp = const.tile([P, 2, ILEN], FP8, tag="sxp")
        img4 = sxp[:].rearrange("p k (r c) -> p k r c", c=PW)
        interiors = [img4[:, kc, PAD:PAD + H, PAD:PAD + W] for kc in range(2)]
        spool = const.tile([P, 2, 16], FP8, tag="spool")  # 16-wide: DR k rule
        xs10 = xload.tile([P, 2, len(XBLKS)], F32, tag="xs10", name="xs10")
        seam0 = (PAD + 1) * PW - PAD
        seams = sxp[:, :, seam0:seam0 + (H - 1) * PW].rearrange(
            "p k (r c) -> p k r c", c=PW)[:, :, :, 0:2 * PAD]

        xsbs = {}
        for bi, (r0, nr) in enumerate(XBLKS):
            for kc in range(2):
                xsbs[(bi, kc)] = xload.tile([P, nr, W], F32, tag=f"xsb{nr}",
                                            name=f"xsb_{bi}_{kc}")
        nc.vector.memset(sxp[:, :, 0:PAD * PW + PAD], 0.0)
        for bi, (r0, nr) in enumerate(XBLKS):
            nc.sync.dma_start(xsbs[(bi, 0)][:], xs.ap()[0:P, r0:r0 + nr])
            nc.gpsimd.dma_start(xsbs[(bi, 1)][:],
                                xs.ap()[P:2 * P, r0:r0 + nr])
            if bi == 0:
                nc.sync.dma_start(lhsT[:, 0:NBLK1], wt.ap()[:, 0:NBLK1])
                nc.gpsimd.memset(seams, 0.0)
            if bi == 1:
                nc.gpsimd.memset(sxp[:, :, (PAD + H) * PW - PAD:ILEN], 0.0)
        for bi, (r0, nr) in enumerate(XBLKS):
            for kc in range(2):
                xsb = xsbs[(bi, kc)]
                nc.vector.reduce_sum(xs10[:, kc, bi:bi + 1], xsb[:],
                                     axis=AX.XY)
                nc.scalar.activation(interiors[kc][:, r0:r0 + nr], xsb[:],
                                     AF.Sign)
        xsum = xload.tile([P, 2], F32, tag="xsum", name="xsum")
        nc.vector.reduce_sum(xsum[:], xs10[:], axis=AX.X)
        nc.scalar.activation(spool[:, :, 0], xsum[:], AF.Sign)
        nc.sync.dma_start(lhsT[:, NBLK1:], wt.ap()[:, NBLK1:])

        # per-group stats [P, S, 2] = (mean, var) per slot
        stats_g = [stat.tile([P, len(m), 2], F32, tag=f"stats_g{g}",
                             name=f"stats_g{g}") for g, m in enumerate(GROUPS)]
        # bn_stats accumulators per branch
        bn6 = {j: stat.tile([P, 2, NTILE, 6], F32, tag=f"bn6_{j}",
                            name=f"bn6_{j}") for j in BR}
        s0 = stat.tile([P, 2], F32, tag="s0", name="s0")
        nc.vector.memset(s0[:], 0.0)

        y16_all = {j: {mc: ybuf.tile([P, HW], F16, tag=f"y{j}_{mc}",
                                     name=f"y{j}_{mc}") for mc in range(2)}
                   for j in BR}
        q = {mc: y16_all[2][mc] for mc in range(2)}  # r1's tiles hold the sum
        ypool = {}
        coefs = {}
        gathers = {}

        def emit_unit(j, mc, t):
            tap0, r = BR[j]
            taps = ([(tap0, 1, 1)] if r is None else
                    [(tap0 + 3 * ky + kx, ky, kx)
                     for ky in range(3) for kx in range(3)])
            h0 = TROWS * t
            acc = ppool.tile([P, TROWS * W], F32, tag="acc")
            for i_mm, (tap, ky, kx) in enumerate(taps):
                rr = 0 if r is None else r
                rbase = PAD + h0 + rr * (ky - 1)
                cbase = PAD + rr * (kx - 1)
                rhs = img4[:, :, rbase:rbase + TROWS, cbase:cbase + W]
                nc.tensor.matmul(acc[:], wdr(tap * 2 + mc), rhs,
                                 start=(i_mm == 0),
                                 stop=(i_mm == len(taps) - 1),
                                 perf_mode=DR)
            ysl = y16_all[j][mc][:, h0 * W:(h0 + TROWS) * W]
            nc.scalar.activation(ysl, acc[:], AF.Copy)
            nc.vector.bn_stats(bn6[j][:, mc, t], ysl)
            if t == NTILE - 1:
                g, si = SLOT[(j, mc)]
                nc.vector.bn_aggr(stats_g[g][:, si], bn6[j][:, mc])

        def emit_pool_branch():
            for mc in range(2):
                yp = psmall.tile([P, 1], F32, tag="yp")
                nc.tensor.matmul(yp[:], wdr(0 * 2 + mc), spool[:, :, 0:1],
                                 start=True, stop=True, perf_mode=DR)
                ys = stat.tile([P, 1], F32, tag=f"ypool{mc}",
                               name=f"ypool{mc}")
                nc.scalar.activation(ys[:], yp[:], AF.Copy)
                ypool[mc] = ys
                g, si = SLOT[(0, mc)]
                nc.vector.tensor_copy(stats_g[g][:, si, 0:1], ys[:])
                nc.vector.memset(stats_g[g][:, si, 1:2], 0.0)

        def issue_gather(g):
            S = len(GROUPS[g])
            st_in = dram.tile([P, S * 2], F32, tag=f"st_in{g}")
            st_out = dram.tile([8, P, S * 2], F32, tag=f"st_out{g}",
                               addr_space="Shared" if n_cores > 4 else "Local")
            nc.sync.dma_start(st_in[:], stats_g[g][:])
            nc.gpsimd.collective_compute(
                "AllGather", ALU.bypass,
                replica_groups=[list(range(n_cores))],
                ins=[st_in[:].opt()], outs=[st_out[:].opt()],
            )
            gathers[g] = st_out

        def fetch_gather(g):
            """Readback DMA, emitted separately so it never head-of-line
            blocks a later group's st_in on the sync queue."""
            S = len(GROUPS[g])
            gath = stat.tile([P, 8, S * 2], F32, tag=f"gath{g}",
                             name=f"gath{g}")
            nc.sync.dma_start(gath[:],
                              gathers[g][:].rearrange("c p f -> p c f"))
            gathers[g] = gath

        def emit_merge(g, on_pool):
            """Cross-core moment merge + (a, lo, hi) coefs for group g."""
            ve = nc.gpsimd if on_pool else nc.vector
            S = len(GROUPS[g])
            F = S * 2
            gath = gathers[g]
            t4 = tmp.tile([P, 4, F], F32, tag=f"t4_{g}", name=f"t4_{g}")
            ve.tensor_tensor(t4[:], gath[:, 0:4], gath[:, 4:8], op=ALU.add)
            t2 = tmp.tile([P, 2, F], F32, tag=f"t2_{g}", name=f"t2_{g}")
            ve.tensor_tensor(t2[:], t4[:, 0:2], t4[:, 2:4], op=ALU.add)
            t1 = tmp.tile([P, F], F32, tag=f"t1_{g}", name=f"t1_{g}")
            ve.tensor_tensor(t1[:], t2[:, 0], t2[:, 1], op=ALU.add)
            gmean = gath[:].rearrange("p c (s f) -> p c s f", f=2)[:, :, :, 0]
            sq = tmp.tile([P, 8, S], F32, tag=f"sq_{g}", name=f"sq_{g}")
            ve.tensor_tensor(sq[:], gmean, gmean, op=ALU.mult)
            q4 = tmp.tile([P, 4, S], F32, tag=f"q4_{g}", name=f"q4_{g}")
            ve.tensor_tensor(q4[:], sq[:, 0:4], sq[:, 4:8], op=ALU.add)
            q2 = tmp.tile([P, 2, S], F32, tag=f"q2_{g}", name=f"q2_{g}")
            ve.tensor_tensor(q2[:], q4[:, 0:2], q4[:, 2:4], op=ALU.add)
            q1 = tmp.tile([P, S], F32, tag=f"q1_{g}", name=f"q1_{g}")
            ve.tensor_tensor(q1[:], q2[:, 0], q2[:, 1], op=ALU.add)

            t1v = t1[:].rearrange("p (s f) -> p s f", f=2)
            means = t1v[:, :, 0]
            vars_ = t1v[:, :, 1]
            mu = tmp.tile([P, S], F32, tag=f"mu{g}", name=f"mu{g}")
            ve.tensor_scalar(mu[:], means, 1.0 / n_cores, None, op0=ALU.mult)
            var = tmp.tile([P, S], F32, tag=f"var{g}", name=f"var{g}")
            ve.tensor_tensor(var[:], vars_, q1[:], op=ALU.add)
            ve.tensor_scalar(var[:], var[:], 1.0 / n_cores, None, op0=ALU.mult)
            musq = tmp.tile([P, S], F32, tag=f"musq{g}", name=f"musq{g}")
            ve.tensor_tensor(musq[:], mu[:], mu[:], op=ALU.mult)
            ve.tensor_tensor(var[:], var[:], musq[:], op=ALU.subtract)
            ve.tensor_scalar(var[:], var[:], EPS, None, op0=ALU.add)
            std = tmp.tile([P, S], F32, tag=f"std{g}", name=f"std{g}")
            nc.scalar.activation(std[:], var[:], AF.Sqrt)
            inv = tmp.tile([P, S], F32, tag=f"inv{g}", name=f"inv{g}")
            nc.vector.reciprocal(inv[:], std[:])   # DVE-only op
            off = GOFF[g]
            gam = gb_sb[:, off:off + S, 0]
            bet = gb_sb[:, off:off + S, 1]
            a_t = stat.tile([P, S], F32, tag=f"a{g}", name=f"a{g}")
            lo_t = stat.tile([P, S], F32, tag=f"lo{g}", name=f"lo{g}")
            hi_t = stat.tile([P, S], F32, tag=f"hi{g}", name=f"hi{g}")
            d_ = tmp.tile([P, S], F32, tag=f"d{g}", name=f"d{g}")
            ve.tensor_tensor(a_t[:], gam, inv[:], op=ALU.mult)
            ve.tensor_tensor(d_[:], mu[:], a_t[:], op=ALU.mult)
            ve.tensor_tensor(d_[:], bet, d_[:], op=ALU.subtract)
            inva = tmp.tile([P, S], F32, tag=f"inva{g}", name=f"inva{g}")
            nc.vector.reciprocal(inva[:], a_t[:])
            ve.tensor_scalar(lo_t[:], d_[:], -1.0, -1.0,
                             op0=ALU.mult, op1=ALU.add)
            ve.tensor_tensor(lo_t[:], lo_t[:], inva[:], op=ALU.mult)
            ve.tensor_scalar(hi_t[:], d_[:], -1.0, 1.0,
                             op0=ALU.mult, op1=ALU.add)
            ve.tensor_tensor(hi_t[:], hi_t[:], inva[:], op=ALU.mult)
            # s0 += per-mc sums of d_j (+ the pool branch's a*y_pool)
            if g == 0:
                dsum = tmp.tile([P, 2], F32, tag="dsum0", name="dsum0")
                dv = d_[:].rearrange("p (m s) -> p m s", m=2)
                ve.tensor_tensor(dsum[:], dv[:, :, 0], dv[:, :, 1], op=ALU.add)
                ve.tensor_tensor(dsum[:], dsum[:], dv[:, :, 2], op=ALU.add)
                ve.tensor_tensor(s0[:], s0[:], dsum[:], op=ALU.add)
                # scalar_tensor_tensor is DVE-only
                nc.vector.scalar_tensor_tensor(s0[:, 0:1], ypool[0][:],
                                               a_t[:, 0:1], s0[:, 0:1],
                                               op0=ALU.mult, op1=ALU.add)
                nc.vector.scalar_tensor_tensor(s0[:, 1:2], ypool[1][:],
                                               a_t[:, 3:4], s0[:, 1:2],
                                               op0=ALU.mult, op1=ALU.add)
            elif g in (1, 2):
                ve.tensor_tensor(s0[:], s0[:], d_[:], op=ALU.add)
            else:
                mc = GROUPS[g][0][1]
                ve.tensor_tensor(s0[:, mc:mc + 1], s0[:, mc:mc + 1],
                                 d_[:, 0:1], op=ALU.add)
            coefs[g] = dict(a=a_t, lo=lo_t, hi=hi_t)

        def apply_mid(j, mcs=(0, 1)):
            """clip+scale branch j, accumulating into q (r1 scales in
            place -- its tiles ARE q)."""
            for mc in mcs:
                g, si = SLOT[(j, mc)]
                c = coefs[g]
                yt = y16_all[j][mc]
                nc.vector.tensor_scalar(yt[:], yt[:], c["lo"][:, si:si + 1],
                                        c["hi"][:, si:si + 1],
                                        op0=ALU.max, op1=ALU.min)
                if j == 2:
                    nc.vector.tensor_scalar(yt[:], yt[:], c["a"][:, si:si + 1],
                                            None, op0=ALU.mult)
                else:
                    nc.vector.scalar_tensor_tensor(
                        q[mc][:], yt[:], c["a"][:, si:si + 1], q[mc][:],
                        op0=ALU.mult, op1=ALU.add)

        def apply_tail(mc):
            """r12 branch half: sf = (a*clip(y) + s0) + q, fp16 out tiles,
            interleaved with the two output DMA chunks."""
            g, si = SLOT[(5, mc)]
            c = coefs[g]
            yt = y16_all[5][mc]
            nc.vector.tensor_scalar(yt[:], yt[:], c["lo"][:, si:si + 1],
                                    c["hi"][:, si:si + 1],
                                    op0=ALU.max, op1=ALU.min)
            nc.vector.tensor_scalar(yt[:], yt[:], c["a"][:, si:si + 1],
                                    s0[:, mc:mc + 1],
                                    op0=ALU.mult, op1=ALU.add)
            for t in range(2):
                sf = sbout.tile([P, 2048], F16, tag="sf")
                nc.vector.tensor_tensor(sf[:], yt[:, t * 2048:(t + 1) * 2048],
                                        q[mc][:, t * 2048:(t + 1) * 2048],
                                        op=ALU.add)
                deng = nc.sync if t == 0 else nc.scalar
                deng.dma_start(
                    out.ap()[mc * P:(mc + 1) * P].rearrange(
                        "m h w -> m (h w)")[:, t * 2048:(t + 1) * 2048],
                    sf[:])

        # ---- emission ------------------------------------------------
        # Scheduling shape (t = approx. time each lands on HW):
        #   S0 (1x1+r1) ends ~48us -> G0 issued; cc blocks Pool, so
        #   merge-G0 runs on Pool in its free window (~66); the DVE
        #   applies are emitted only after G1's issue so S1's bn_stats /
        #   aggr chain (which gates G1) is never queued behind them.
        #   Same pattern for S2/G1.  From S3 on, Pool is permanently
        #   cc-blocked: merges move to DVE.  r12's stats gathers are
        #   split per mc: mc0's gather rides under S4's matmuls and
        #   mc0's output DMA ships inside mc1's gather window, leaving
        #   only merge-G4 + the mc1 tail passes exposed.
        # S0: 1x1 + r1 interleaved (1x1 tile 0 first: needs only 8 rows)
        units_x = [(1, mc, t) for mc in range(2) for t in range(NTILE)]
        units_r1 = [(2, mc, t) for mc in range(2) for t in range(NTILE)]
        order = [units_x[0], units_x[NTILE]]   # 1x1 mc0 t0, mc1 t0
        rest_x = units_x[1:NTILE] + units_x[NTILE + 1:]
        for i, u in enumerate(units_r1):
            order.append(u)
            if i < len(rest_x):
                order.append(rest_x[i])
        for u in order:
            emit_unit(*u)
        emit_pool_branch()
        issue_gather(0)

        # S1: r4
        for mc in range(2):
            for t in range(NTILE):
                emit_unit(3, mc, t)
        fetch_gather(0)
        emit_merge(0, on_pool=True)      # Pool, runs ~66 between cc blocks
        issue_gather(1)
        apply_mid(2)                     # DVE, runs during S2's stream
        apply_mid(1)

        # S2: r8
        for mc in range(2):
            for t in range(NTILE):
                emit_unit(4, mc, t)
        fetch_gather(1)
        emit_merge(1, on_pool=True)      # Pool, runs ~98
        issue_gather(2)
        apply_mid(3)                     # DVE, runs during S3's stream

        # S3: r12 mc0
        for t in range(NTILE):
            emit_unit(5, 0, t)
        issue_gather(3)
        fetch_gather(2)
        emit_merge(2, on_pool=False)     # DVE (Pool is cc-blocked now)
        apply_mid(4, mcs=(0,))           # r8 mc0 before S4's stats chain

        # S4: r12 mc1
        for t in range(NTILE):
            emit_unit(5, 1, t)
        issue_gather(4)
        apply_mid(4, mcs=(1,))           # r8 mc1 inside the G4 window
        fetch_gather(3)
        emit_merge(3, on_pool=False)
        apply_tail(0)                    # mc0 output ships in G4's window
        fetch_gather(4)
        emit_merge(4, on_pool=False)
        apply_tail(1)

    nc.compile()
    return nc


def pack_weights(w_pool, w1, w3):
    """Host filter transform: sign -> DoubleRow k-interleave, fp8.

    wt[k, t*2+mc, i, m] = sign(W_t[mc*128+m, i*128+k]); block (t*2+mc) is
    the stationary [2, 128] operand for logical tap t / out-chunk mc.
    """
    mats = [np.sign(np.asarray(w_pool, np.float32).reshape(COUT, CIN)),
            np.sign(np.asarray(w1, np.float32).reshape(COUT, CIN))]
    w3 = np.asarray(w3, np.float32)
    for i in range(len(RATES)):
        for ky in range(3):
            for kx in range(3):
                mats.append(np.sign(w3[i, :, :, ky, kx]))
    wt = np.zeros((P, NT * 2, 2, P), np.float32)  # [k, blk, i, m]
    for t, m in enumerate(mats):
        for mc in range(2):
            for i in range(2):
                blk = m[mc * P:(mc + 1) * P, i * P:(i + 1) * P]   # [m, k]
                wt[:, t * 2 + mc, i, :] = blk.T
    return wt.astype(mybir.dt.np(FP8))


def pack_gb(g_pool, b_pool, g1, b1, g3, b3):
    """gamma/beta packed [P, slot, 2] in global gather-slot order."""
    gs = [g_pool, g1] + [g3[i] for i in range(len(RATES))]
    bs = [b_pool, b1] + [b3[i] for i in range(len(RATES))]
    slots = [jm for members in GROUPS for jm in members]
    gb = np.zeros((P, 12, 2), np.float32)
    for s, (j, mc) in enumerate(slots):
        gb[:, s, 0] = np.asarray(gs[j], np.float32)[mc * P:(mc + 1) * P]
        gb[:, s, 1] = np.asarray(bs[j], np.float32)[mc * P:(mc + 1) * P]
    return gb


_NC = None


def _get_nc():
    global _NC
    if _NC is None:
        _NC = build(N_CORES)
    return _NC


def make_in_maps(x, w_pool, g_pool, b_pool, w1, g1, b1, w3, g3, b3):
    x = np.asarray(x, np.float32)
    wt = pack_weights(w_pool, w1, w3)
    gb = pack_gb(g_pool, b_pool, g1, b1, g3, b3)
    return [
        {"xs": np.ascontiguousarray(x[c]), "wt": wt, "gb": gb}
        for c in range(x.shape[0])
    ]


def kernel(x, w_pool, g_pool, b_pool, w1, g1, b1, w3, g3, b3):
    nc = _get_nc()
    in_maps = make_in_maps(x, w_pool, g_pool, b_pool, w1, g1, b1, w3, g3, b3)
    res = run_bass_kernel_spmd(nc, in_maps, core_ids=list(range(N_CORES)))
    return np.stack([res.results[c]["out"] for c in range(N_CORES)],
                    axis=0).astype(np.float32)
